# revision 1
# baseline (speedup 1.0000x reference)
"""Multi-label masked-gather mean loss on 8 Trainium2 NeuronCores.

reference:
    logp = log_softmax(x, -1); per_sample = -sum_t(mask*logp[i, y[i,t]])/count_i
    loss = mean(per_sample)

Identity used (count_i > 0):
    per_sample_i = logsumexp(x_i) - sum_t w[i,t] * x[i, y[i,t]],  w = mask/count
    loss = (sum_i logsumexp(x_i) + sum_{i,t} wneg[i,t] * x[i,y[i,t]]) / B
with wneg = -w. Data-parallel over the batch: 4096 rows -> 512 rows/core.

Per core the Bass kernel streams its x shard [512, 50257] f32 once from HBM
(memory-bound), computing exp + row-sum via ScalarE activation accumulate,
logsumexp per row, an indirect-DMA gather of the 8 labeled logits per row,
and reduces everything to a per-partition partial sum [128, 1].
Host sums the 8x128 partials and divides by B.
"""

import sys

sys.path.insert(0, "/opt/trn_rl_repo")

import math

import numpy as np

import concourse.bass as bass
import concourse.tile as tile
from concourse import bacc, mybir
from concourse import bass_utils

# Problem shape (hardcoded per contract)
B, C, T = 4096, 50257, 8
NCORES = 8
BL = B // NCORES  # 512 rows per core
P = 128
RB = BL // P      # 4 row blocks per core
CW = 16384        # column tile width (bf16 -> 32 KiB per partition)
GCOLS = BL * T // P      # 32: gathered elements per partition


MAXW = 17489                       # widest tile (pool slot size)


def _col_tiles(rb):
    """One tile per (DMA, ACT) piece — a tile is never simultaneously
    read by ACT and written by a later DMA (intra-tile sharing measured
    ~20% ACT slowdown). Row block 0 ramps up so ACT starts ~11us in and
    DMA (only ~1.3x ACT's rate) never falls behind after the start."""
    if rb == 0:
        widths = [2048, 4096, 6144, 9216, 12288, 16465]
    else:
        widths = [16384, 16384, 17489]
    tiles = []
    c0 = 0
    for w in widths:
        tiles.append((c0, w))
        c0 += w
    assert c0 == C
    return tiles


_NCT_BY_RB = [len(_col_tiles(rb)) for rb in range(RB)]
ACC_COLS = sum(_NCT_BY_RB)         # per-ACT-piece sumexp cols
OUT_COLS = ACC_COLS + 1            # + gather-dot col

_f32 = mybir.dt.float32
_bf16 = mybir.dt.bfloat16
_i32 = mybir.dt.int32

_compiled = None  # (nc, names) cache


def _build():
    nc = bacc.Bacc(
        "TRN2",
        target_bir_lowering=False,
        debug=False,
        enable_asserts=False,
        num_devices=NCORES,
    )
    x_t = nc.dram_tensor("x", [BL, C], _bf16, kind="ExternalInput")
    idx_t = nc.dram_tensor("idx", [P, GCOLS], _i32, kind="ExternalInput")
    wneg_t = nc.dram_tensor("wneg", [P, GCOLS], _f32, kind="ExternalInput")
    # cols 0..ACC_COLS-1: per-(rowblock, coltile) sumexp partials;
    # col ACC_COLS: sum_t wneg*gathered. Host sums + logs.
    out_t = nc.dram_tensor("out", [P, OUT_COLS], _f32, kind="ExternalOutput")

    x = x_t.ap()
    idx = idx_t.ap()
    wneg = wneg_t.ap()
    out = out_t.ap()

    with tile.TileContext(nc) as tc:
        with (
            tc.tile_pool(name="xin", bufs=5) as xin_pool,
            tc.tile_pool(name="scratch", bufs=1) as scratch_pool,
            tc.tile_pool(name="stats", bufs=1) as stats_pool,
            tc.tile_pool(name="gather", bufs=1) as gather_pool,
        ):
            # all partials end up here and go out in one DMA
            acc = stats_pool.tile([P, OUT_COLS], _f32)
            # self-made zero bias for Exp: avoids the const-AP preamble load
            bias0 = stats_pool.tile([P, 1], _f32)
            nc.gpsimd.memset(bias0[:], 0.0)

            # exp output scratch: values are unused, only accum_out matters
            # (fp8 keeps it small; the accumulator itself is fp32).
            exp_scratch = scratch_pool.tile([P, MAXW], mybir.dt.float8e4)

            # --- main stream: exp + row-sum via ACT activation accumulate ---
            col = 0
            for rb in range(RB):
                rows = slice(rb * P, (rb + 1) * P)
                for c0, cw in _col_tiles(rb):
                    xt = xin_pool.tile([P, MAXW], _bf16, tag="xt")
                    nc.sync.dma_start(
                        out=xt[:, :cw], in_=x[rows, c0 : c0 + cw]
                    )
                    nc.scalar.activation(
                        out=exp_scratch[:, :cw],
                        in_=xt[:, :cw],
                        func=mybir.ActivationFunctionType.Exp,
                        bias=bias0[:, 0:1],
                        accum_out=acc[:, col : col + 1],
                    )
                    col += 1
            assert col == ACC_COLS

            # --- gather path (tiny; runs in the shadow of the stream on
            # SWDGE/DVE, completes well before the final ACT) ---
            idx_tile = gather_pool.tile([P, GCOLS], _i32)
            nc.gpsimd.dma_start(out=idx_tile[:], in_=idx[:])
            w_tile = gather_pool.tile([P, GCOLS], _f32)
            nc.gpsimd.dma_start(out=w_tile[:], in_=wneg[:])
            g_tile = gather_pool.tile([P, GCOLS], _bf16)
            nc.gpsimd.indirect_dma_start(
                out=g_tile[:],
                out_offset=None,
                in_=x[:],
                in_offset=bass.IndirectOffsetOnAxis(ap=idx_tile[:], axis=1),
            )
            g32 = gather_pool.tile([P, GCOLS], _f32)
            nc.vector.tensor_copy(out=g32[:], in_=g_tile[:])
            gw = gather_pool.tile([P, GCOLS], _f32)
            nc.vector.tensor_tensor(
                out=gw[:], in0=g32[:], in1=w_tile[:], op=mybir.AluOpType.mult
            )
            nc.vector.tensor_reduce(
                out=acc[:, ACC_COLS : ACC_COLS + 1],
                in_=gw[:],
                axis=mybir.AxisListType.X,
                op=mybir.AluOpType.add,
            )

            # out via the scalar engine's HWDGE ring: no cross-engine hop
            # after the last ACT writes its accumulator column.
            nc.scalar.dma_start(out=out[:], in_=acc[:])

    nc.compile()
    return nc


def _get_compiled():
    global _compiled
    if _compiled is None:
        _compiled = _build()
    return _compiled


def _make_in_maps(x, y):
    import ml_dtypes

    # bf16 staging: halves HBM traffic; loss rel err impact ~1e-6 (rounding
    # averages out across 50k-element rows).
    x = np.ascontiguousarray(np.asarray(x, dtype=np.float32).astype(ml_dtypes.bfloat16))
    y = np.asarray(y)
    mask = y != -1
    cnt = mask.sum(axis=1)
    # rows with count 0 would be NaN in the reference; inputs never hit this
    w = np.where(mask, 1.0 / np.maximum(cnt, 1)[:, None], 0.0).astype(np.float32)
    wneg = -w
    safe = np.where(mask, y, 0).astype(np.int64)

    in_maps = []
    for m in range(NCORES):
        sl = slice(m * BL, (m + 1) * BL)
        xs = x[sl]
        flat = (
            np.arange(BL, dtype=np.int64)[:, None] * C + safe[sl]
        ).astype(np.int32)
        in_maps.append(
            {
                "x": xs,
                "idx": np.ascontiguousarray(flat.reshape(P, GCOLS)),
                "wneg": np.ascontiguousarray(wneg[sl].reshape(P, GCOLS)),
            }
        )
    return in_maps


def kernel(**inputs) -> np.ndarray:
    x, y = inputs["x"], inputs["y"]
    nc = _get_compiled()
    in_maps = _make_in_maps(x, y)
    res = bass_utils.run_bass_kernel_spmd(
        nc, in_maps, core_ids=list(range(NCORES))
    )
    total = 0.0
    for r in res.results:
        out = np.asarray(r["out"], dtype=np.float64)  # [P, OUT_COLS]
        col = 0
        for rb in range(RB):
            n = _NCT_BY_RB[rb]
            se = out[:, col : col + n].sum(axis=1)  # per-row sumexp
            total += np.log(se).sum()
            col += n
        total += out[:, ACC_COLS].sum()
    return np.float32(total / B)



# revision 4
# speedup vs baseline: 1.5061x; 1.5061x over previous
"""Multi-label masked-gather mean loss on 8 Trainium2 NeuronCores.

reference:
    logp = log_softmax(x, -1); per_sample = -sum_t(mask*logp[i, y[i,t]])/count_i
    loss = mean(per_sample)

Identity used (count_i > 0):
    per_sample_i = logsumexp(x_i) - sum_t w[i,t] * x[i, y[i,t]],  w = mask/count
    loss = (sum_i logsumexp(x_i) + sum_{i,t} wneg[i,t] * x[i,y[i,t]]) / B
Data-parallel over the batch: 4096 rows -> 512 rows/core.

The heavy part is sum_j exp(x_ij) over C=50257 columns. The baseline ran it
all on the scalar engine (ACT, 1 elem/cycle @ 1.2 GHz -> ~178us busy).
This version splits the columns across three paths so ACT and DVE work
concurrently and the DMA bytes shrink (fp8 staging where possible):

  * ACT path (cols [0, CA), fp8): ScalarE Exp + accum_out, 1 cyc/elem.
    fp8 input quantization shifts E[sum exp] by only ~2e-5 (measured).
  * DVE fast path (cols [CA, CA+CB), bf16): Schraudolph exp approximation
      i16 = rint(A*x + B);  bitcast(i16) as fp16  ~=  exp(x)
    via tensor_scalar bf16->i16 in 4x perf mode (0.25 cyc/elem; HW-verified),
    then a pairwise fp16 add tree (tensor_tensor, 2x mode) + one small
    tensor_reduce for the row sum (~0.5 cyc/elem amortized). accum_out is
    NOT used for the sum: it demotes tensor_scalar to the 1x CACHE_REDUCE
    opcode (HW-measured).
  * DVE slow path (cols [CA+CB, C), fp8): same Schraudolph but the 1-byte
    input forces 1x mode (1 cyc/elem). Still worth a slice: it converts
    scarce DMA bytes into spare DVE cycles.

  B is bias-corrected (15301.09 vs the nominal 15360) so that
  E[approx exp / exp] = 1 under the N(0,1) input distribution; residual
  full-pipeline loss error ~1e-5 relative (numpy-validated, incl. the fp16
  tree rounding), far under the 2e-2 gate. Conversion is round-to-nearest
  (HW-verified), values stay in int16/fp16-safe ranges for |x| <= 11.

GpSimd (Pool) is deliberately NOT used for compute: its SBUF port is shared
with the DVE and concurrent Pool tensor ops halve DVE throughput
(HW-measured). It only runs the tiny indirect gather for the label term.

Per-core per-instruction emission order matches DMA arrival order (engines
execute in-order; a stalled op blocks later ready ops on the same queue).
"""

import sys

sys.path.insert(0, "/opt/trn_rl_repo")

import numpy as np

import concourse.bass as bass
import concourse.tile as tile
from concourse import bacc, mybir
from concourse import bass_utils

# Problem shape (hardcoded per contract)
B, C, T = 4096, 50257, 8
NCORES = 8
BL = B // NCORES  # 512 rows per core
P = 128
RB = BL // P      # 4 row blocks per core
GCOLS = BL * T // P      # 32: gathered elements per partition

# --- column split across engine paths ---
CA = 27649                     # ACT fp8 columns
CB = 16384                     # DVE fast bf16 columns
CD = C - CA - CB               # 6224: DVE slow fp8 columns

# per-row-block tile widths (rb0 ramps up so engines start early)
ACT_TILES = {0: [2048, 4608, 9216, 11777], 1: [13824, 13825]}
FAST_TILES = {0: [4608, 11776], 1: [16384]}
SLOW_TILES = {0: [CD], 1: [CD]}
for d in (ACT_TILES, FAST_TILES, SLOW_TILES):
    d[2] = d[1]
    d[3] = d[1]
assert sum(ACT_TILES[0]) == sum(ACT_TILES[1]) == CA
assert sum(FAST_TILES[0]) == sum(FAST_TILES[1]) == CB

# Schraudolph constants (fp16 domain), bias-corrected for N(0,1) inputs
SCH_A = 1477.3197218702985          # 2^10 / ln 2
SCH_B_BF16 = 15301.091
SCH_B_FP8 = 15301.093
TREE_MIN = 512                      # stop pairwise halving at this width

ACT_MAXW = max(max(v) for v in ACT_TILES.values())
FAST_MAXW = max(max(v) for v in FAST_TILES.values())

# accumulator column layout: per rb, [ACT tiles..., fast tiles..., slow tile],
# then one final gather column
_COLS_PER_RB = [len(ACT_TILES[rb]) + len(FAST_TILES[rb]) + len(SLOW_TILES[rb])
                for rb in range(RB)]
ACC_COLS = sum(_COLS_PER_RB)
OUT_COLS = ACC_COLS + 1

_f32 = mybir.dt.float32
_f16 = mybir.dt.float16
_bf16 = mybir.dt.bfloat16
_i16 = mybir.dt.int16
_i32 = mybir.dt.int32
_f8 = mybir.dt.float8e4

_compiled = None


def _build():
    nc = bacc.Bacc(
        "TRN2",
        target_bir_lowering=False,
        debug=False,
        enable_asserts=False,
        num_devices=NCORES,
    )
    x8_t = nc.dram_tensor("x8", [BL, C], _f8, kind="ExternalInput")
    xb_t = nc.dram_tensor("xb", [BL, CB], _bf16, kind="ExternalInput")
    idx_t = nc.dram_tensor("idx", [P, GCOLS], _i32, kind="ExternalInput")
    wneg_t = nc.dram_tensor("wneg", [P, GCOLS], _f32, kind="ExternalInput")
    out_t = nc.dram_tensor("out", [P, OUT_COLS], _f32, kind="ExternalOutput")

    x8 = x8_t.ap()
    xb = xb_t.ap()
    idx = idx_t.ap()
    wneg = wneg_t.ap()
    out = out_t.ap()

    mult = mybir.AluOpType.mult
    add = mybir.AluOpType.add

    with tile.TileContext(nc) as tc:
        with (
            tc.tile_pool(name="actin", bufs=3) as actin_pool,
            tc.tile_pool(name="fast", bufs=3) as fast_pool,
            tc.tile_pool(name="slow8", bufs=2) as slow8_pool,
            tc.tile_pool(name="slow16", bufs=2) as slow16_pool,
            tc.tile_pool(name="scratch", bufs=1) as scratch_pool,
            tc.tile_pool(name="stats", bufs=1) as stats_pool,
            tc.tile_pool(name="gather", bufs=1) as gather_pool,
        ):
            acc = stats_pool.tile([P, OUT_COLS], _f32)
            bias0 = stats_pool.tile([P, 1], _f32)
            nc.gpsimd.memset(bias0[:], 0.0)

            # ACT exp output scratch: values unused, only accum_out matters
            scr8 = scratch_pool.tile([P, ACT_MAXW], mybir.dt.float8e4)

            # gather inputs early (SWDGE, overlaps the stream)
            idx_tile = gather_pool.tile([P, GCOLS], _i32)
            nc.gpsimd.dma_start(out=idx_tile[:], in_=idx[:])
            w_tile = gather_pool.tile([P, GCOLS], _f32)
            nc.gpsimd.dma_start(out=w_tile[:], in_=wneg[:])
            g_tile = gather_pool.tile([P, GCOLS], _f8)
            nc.gpsimd.indirect_dma_start(
                out=g_tile[:],
                out_offset=None,
                in_=x8[:],
                in_offset=bass.IndirectOffsetOnAxis(ap=idx_tile[:], axis=1),
            )

            col = 0
            for rb in range(RB):
                rows = slice(rb * P, (rb + 1) * P)
                acts = ACT_TILES[rb]
                fasts = FAST_TILES[rb]

                # column base offsets
                act_off = [0]
                for w in acts[:-1]:
                    act_off.append(act_off[-1] + w)
                fast_off = [0]
                for w in fasts[:-1]:
                    fast_off.append(fast_off[-1] + w)

                cols_act = list(range(col, col + len(acts)))
                cols_fast = list(range(col + len(acts), col + len(acts) + len(fasts)))
                col_slow = col + len(acts) + len(fasts)
                col += _COLS_PER_RB[rb]

                # ---- DMA + compute emission, interleaved so the DMA queue
                # feeds ACT and DVE as evenly as possible ----
                # 1) first fast tile (DVE starts earliest)
                w0 = fasts[0]
                ft0 = fast_pool.tile([P, FAST_MAXW], _bf16, tag="fast")
                nc.sync.dma_start(out=ft0[:, :w0], in_=xb[rows, 0:w0])

                # 2) ACT tiles 0..1
                act_tiles_sb = []
                for i in range(len(acts)):
                    at = actin_pool.tile([P, ACT_MAXW], _f8, tag="act")
                    act_tiles_sb.append(at)
                ai = 0

                def emit_act_dma(i):
                    nc.sync.dma_start(
                        out=act_tiles_sb[i][:, : acts[i]],
                        in_=x8[rows, act_off[i] : act_off[i] + acts[i]],
                    )

                def emit_act_compute(i):
                    nc.scalar.activation(
                        out=scr8[:, : acts[i]],
                        in_=act_tiles_sb[i][:, : acts[i]],
                        func=mybir.ActivationFunctionType.Exp,
                        bias=bias0[:, 0:1],
                        accum_out=acc[:, cols_act[i] : cols_act[i] + 1],
                    )

                emit_act_dma(0)
                emit_act_dma(1)

                # 3) second fast tile DMA (if any)
                ft1 = None
                if len(fasts) > 1:
                    w1 = fasts[1]
                    ft1 = fast_pool.tile([P, FAST_MAXW], _bf16, tag="fast")
                    nc.sync.dma_start(
                        out=ft1[:, :w1], in_=xb[rows, fast_off[1] : fast_off[1] + w1]
                    )

                # 4) remaining ACT DMAs + slow DMA
                for i in range(2, len(acts)):
                    emit_act_dma(i)
                st8 = slow8_pool.tile([P, CD], _f8, tag="slow8")
                nc.sync.dma_start(out=st8[:], in_=x8[rows, CA + CB : C])

                # ---- compute emission ----
                for i in range(len(acts)):
                    emit_act_compute(i)

                # DVE: fast tiles then slow tile (arrival order)
                def fast_chain(ft, w, c):
                    sch_b = SCH_B_BF16
                    nc.vector.tensor_scalar(
                        out=ft[:, :w].bitcast(_i16), in0=ft[:, :w],
                        scalar1=SCH_A, scalar2=sch_b, op0=mult, op1=add,
                    )
                    f16 = ft[:].bitcast(_f16)
                    n = w
                    while n > TREE_MIN and n % 2 == 0:
                        h = n // 2
                        nc.vector.tensor_tensor(
                            out=f16[:, :h], in0=f16[:, :h], in1=f16[:, h:n], op=add
                        )
                        n = h
                    nc.vector.tensor_reduce(
                        out=acc[:, c : c + 1], in_=f16[:, :n],
                        axis=mybir.AxisListType.X, op=add,
                    )

                fast_chain(ft0, fasts[0], cols_fast[0])
                if ft1 is not None:
                    fast_chain(ft1, fasts[1], cols_fast[1])

                # slow: fp8 -> i16 (1x), then fp16 tree
                so = slow16_pool.tile([P, CD], _i16, tag="slow16")
                nc.vector.tensor_scalar(
                    out=so[:], in0=st8[:], scalar1=SCH_A, scalar2=SCH_B_FP8,
                    op0=mult, op1=add,
                )
                f16 = so[:].bitcast(_f16)
                n = CD
                while n > TREE_MIN and n % 2 == 0:
                    h = n // 2
                    nc.vector.tensor_tensor(
                        out=f16[:, :h], in0=f16[:, :h], in1=f16[:, h:n], op=add
                    )
                    n = h
                nc.vector.tensor_reduce(
                    out=acc[:, col_slow : col_slow + 1], in_=f16[:, :n],
                    axis=mybir.AxisListType.X, op=add,
                )

            # ---- gather tail (tiny, on DVE after the streams) ----
            g32 = gather_pool.tile([P, GCOLS], _f32)
            nc.vector.tensor_copy(out=g32[:], in_=g_tile[:])
            gw = gather_pool.tile([P, GCOLS], _f32)
            nc.vector.tensor_tensor(
                out=gw[:], in0=g32[:], in1=w_tile[:], op=mult
            )
            nc.vector.tensor_reduce(
                out=acc[:, ACC_COLS : ACC_COLS + 1],
                in_=gw[:],
                axis=mybir.AxisListType.X,
                op=add,
            )

            # out via the scalar engine's HWDGE ring
            nc.scalar.dma_start(out=out[:], in_=acc[:])

    nc.compile()
    return nc


def _get_compiled():
    global _compiled
    if _compiled is None:
        _compiled = _build()
    return _compiled


def _make_in_maps(x, y):
    import ml_dtypes

    xf = np.asarray(x, dtype=np.float32)
    y = np.asarray(y)
    mask = y != -1
    cnt = mask.sum(axis=1)
    # rows with count 0 would be NaN in the reference; inputs never hit this
    w = np.where(mask, 1.0 / np.maximum(cnt, 1)[:, None], 0.0).astype(np.float32)
    wneg = -w
    safe = np.where(mask, y, 0).astype(np.int64)

    in_maps = []
    for m in range(NCORES):
        sl = slice(m * BL, (m + 1) * BL)
        xs = xf[sl]
        x8 = np.ascontiguousarray(xs).astype(ml_dtypes.float8_e4m3)
        xbf = np.ascontiguousarray(xs[:, CA : CA + CB]).astype(ml_dtypes.bfloat16)
        flat = (
            np.arange(BL, dtype=np.int64)[:, None] * C + safe[sl]
        ).astype(np.int32)
        in_maps.append(
            {
                "x8": x8,
                "xb": xbf,
                "idx": np.ascontiguousarray(flat.reshape(P, GCOLS)),
                "wneg": np.ascontiguousarray(wneg[sl].reshape(P, GCOLS)),
            }
        )
    return in_maps


def kernel(**inputs) -> np.ndarray:
    x, y = inputs["x"], inputs["y"]
    nc = _get_compiled()
    in_maps = _make_in_maps(x, y)
    res = bass_utils.run_bass_kernel_spmd(
        nc, in_maps, core_ids=list(range(NCORES))
    )
    total = 0.0
    for r in res.results:
        o = np.asarray(r["out"], dtype=np.float64)  # [P, OUT_COLS]
        c0 = 0
        for rb in range(RB):
            n = _COLS_PER_RB[rb]
            se = o[:, c0 : c0 + n].sum(axis=1)  # per-row sumexp
            total += np.log(se).sum()
            c0 += n
        total += o[:, ACC_COLS].sum()
    return np.float32(total / B)
